# revision 18
# baseline (speedup 1.0000x reference)
"""Trainium2 Bass kernel for nn_ModelDEP (biaffine-ish dependency parser loss).

Contract: kernel(**inputs) takes FULL unsharded numpy inputs (as produced by
reference.setup_inputs()) and returns the FULL output (scalar f32 loss).

Strategy (hardcoded, self-contained):
  - Data parallel over batch: B=16 examples -> 8 cores x 2 examples.
  - Per example, on device:
      hidden_T = relu(W1.T @ ctx_T + b1)            [256h x 128i] (h on partitions)
      cwr_T    = [root | hidden_T]                  [256h x 129j]
      ha_T     = Wa.T @ hidden_T + bp               [256 x 128]   (bp folded here)
      cbb_T    = Wb.T @ cwr_T                       [256 x 129]
      arc[i,j] = W_arc . relu(ha_T[:,i] + cbb_T[:,j])
        - per (j, h-chunk): one fused (add bias, max 0) op -> bf16 [128,128] tile
          split between DVE tensor_scalar (~180ns/tile effective) and ACT
          activation-relu (~341ns/tile effective), 26:14 greedy interleave.
          GPSIMD measured ~2µs/tile and regresses the kernel - do not use it.
        - TensorE: lhsT = pairs tile (stationary), rhs = W_arc chunk
          -> PSUM column [128i, 1], accumulated over the 2 h-chunks.
      arc CE: logits are tiny (|x| < 0.15) so no max-subtraction: one Exp with
        accum (sum over j); gold logit via (iota == gold) * logits.  The final
        ln(sum) - gold combine happens on host (tiny [128,8] tensor), so the
        kernel needs only Relu+Exp -> a single act-table load at start.
      label path (no DRAM round-trip): sel_pre[h,i] = ha[h,i] + cbb[gold_i,h]
        computed purely on PE: identity @ ha + cbb_jh(=cj) @ onehot(gold) +
        cbb_root @ onehot(gold==root), accumulated in PSUM; relu on DVE;
        label logits = sel_T.T @ W_lab + b_lab.  All-SBUF, so nothing ever
        blocks an in-order engine queue.
  - Few, large input DMAs (ctx as one [128, 512] transfer per example; weights
    as two [128, 4, 256] transfers on the gpsimd SWDGE queue).
  - Host: ln(es)-gold combine, mask by sentence length, global sum, /denom,
    *0.5.
"""

import sys
import numpy as np

for _p in ("/opt/trn_rl_repo", "/root/.axon_site/_ro/trn_rl_repo"):
    if _p not in sys.path:
        sys.path.append(_p)

import ml_dtypes

import concourse.bass as bass
from concourse import bacc
import concourse.mybir as mybir
import concourse.tile as tile
from concourse.bass_utils import run_bass_kernel_spmd
from concourse.masks import make_identity

BF16 = mybir.dt.bfloat16
F32 = mybir.dt.float32
AF = mybir.ActivationFunctionType
ALU = mybir.AluOpType

B, L, D, H, TAGS = 16, 128, 512, 256, 45
NC_CORES = 8
NB = B // NC_CORES  # examples per core
J = L + 1  # head candidates (root + tokens)
HC = H // 128  # h chunks
DC = D // 128  # d chunks

_nb = ml_dtypes.bfloat16

_cached = {}

# j-loop relu engine split by (j*HC+hc) % 40, assigned greedily by measured
# effective sustained rates (engine time + sem/queue overhead inflation).
def _make_split():
    # effective measured rates incl. per-engine overhead inflation
    rate = {"d": 180.0, "a": 341.0}
    t = {"d": 0.0, "a": 0.0}
    slots = []
    for _ in range(40):
        e = min(rate, key=lambda x: t[x] + rate[x])
        t[e] += rate[e]
        slots.append(e)
    return slots


SPLIT40 = _make_split()

PKF_W = 8 + J + TAGS  # pke (8) | iota row (J) | b_lab rows (TAGS)


def _build_program():
    nc = bacc.Bacc("TRN2", target_bir_lowering=False, debug=False, num_devices=NC_CORES)

    # ---- I/O ----
    ctx_d = nc.dram_tensor("ctx_bf", [NB, 128, DC * 128], BF16, kind="ExternalInput")
    w1_d = nc.dram_tensor("w1_bf", [128, DC, H], BF16, kind="ExternalInput")
    wab_d = nc.dram_tensor("wab_bf", [128, 2 * HC, H], BF16, kind="ExternalInput")
    pkf_d = nc.dram_tensor("pkf_f32", [128, PKF_W], F32, kind="ExternalInput")
    pkb_d = nc.dram_tensor("pack_bf", [128, 4 + HC * TAGS], BF16, kind="ExternalInput")
    oh_d = nc.dram_tensor("oh_bf", [128, NB * 2 * 128], BF16, kind="ExternalInput")
    esg_d = nc.dram_tensor("esg_out", [128, 4 * NB], F32, kind="ExternalOutput")

    with tile.TileContext(nc) as tc:
        with (
            tc.tile_pool(name="consts", bufs=1) as consts,
            tc.tile_pool(name="bpool", bufs=2) as bpool,
            tc.tile_pool(name="pairs", bufs=96) as pairs_pool,
            tc.tile_pool(name="ps_big", bufs=2, space="PSUM") as ps_big,
            tc.tile_pool(name="ps_work", bufs=2, space="PSUM") as ps_work,
            tc.tile_pool(name="ps_lab", bufs=2, space="PSUM") as ps_lab,
        ):
            # ---- critical-path DMAs: ctx ex0 (sync), weights (gpsimd SWDGE),
            #      packs (scalar) ----
            ctxTs = []
            ctxT0 = bpool.tile([128, DC, 128], BF16, tag="ctxT")
            nc.sync.dma_start(out=ctxT0[:], in_=ctx_d.ap()[0])
            w1_sb = consts.tile([128, DC, H], BF16)
            nc.gpsimd.dma_start(out=w1_sb[:], in_=w1_d.ap())
            wab_sb = consts.tile([128, 2 * HC, H], BF16)
            nc.gpsimd.dma_start(out=wab_sb[:], in_=wab_d.ap())
            pkf_sb = consts.tile([128, PKF_W], F32)
            nc.scalar.dma_start(out=pkf_sb[:], in_=pkf_d.ap())
            pkb_sb = consts.tile([128, 4 + HC * TAGS], BF16)
            nc.scalar.dma_start(out=pkb_sb[:], in_=pkb_d.ap())
            # non-critical loads
            ctxT1 = bpool.tile([128, DC, 128], BF16, tag="ctxT")
            nc.sync.dma_start(out=ctxT1[:], in_=ctx_d.ap()[1])
            oh_sb = consts.tile([128, NB * 2 * 128], BF16)
            nc.sync.dma_start(out=oh_sb[:], in_=oh_d.ap())
            ctxTs.append(ctxT0)
            ctxTs.append(ctxT1)

            # bf16 identity for the ha-into-PSUM matmul; ones row for b_lab
            ident_sb = consts.tile([128, 128], BF16)
            make_identity(nc, ident_sb[:])
            ones1 = consts.tile([1, 128], F32)
            nc.gpsimd.memset(ones1[:], 1.0)

            # pin the act table (exp_and_others: has Exp + Relu) before any Relu
            dummy = consts.tile([1, 1], F32)
            nc.scalar.activation(dummy[:], ones1[0:1, 0:1], AF.Exp)

            # output: es (sumexp) and gold logits, combined on host
            esg = consts.tile([128, 4 * NB], F32)
            es4 = esg[:, 0 : 2 * NB]
            golds4 = esg[:, 2 * NB : 4 * NB]

            wa_sb = wab_sb[:, 0:HC]
            wb_sb = wab_sb[:, HC : 2 * HC]

            for b in range(NB):
                ctxT = ctxTs[b]
                # ---- hidden (into cwr cols 1..128) ----
                cwrT = bpool.tile([128, HC, J], BF16, tag="cwrT")
                for hc in range(HC):
                    nc.vector.tensor_copy(cwrT[:, hc, 0:1], pkb_sb[:, hc : hc + 1])
                for hc in range(HC):
                    phw = ps_work.tile([128, H], F32, tag="work")
                    ph = phw[:, :128]
                    for dc in range(DC):
                        nc.tensor.matmul(
                            ph[:],
                            lhsT=w1_sb[:, dc, hc * 128 : (hc + 1) * 128],
                            rhs=ctxT[:, dc, :],
                            start=(dc == 0),
                            stop=(dc == DC - 1),
                        )
                    nc.vector.tensor_scalar(
                        out=cwrT[:, hc, 1:J],
                        in0=ph[:],
                        scalar1=pkf_sb[:, hc : hc + 1],
                        scalar2=0.0,
                        op0=ALU.add,
                        op1=ALU.max,
                    )
                # ---- ha_T (+bp folded) ----
                haT = bpool.tile([128, HC, 128], BF16, tag="haT")
                for ac in range(HC):
                    paw = ps_work.tile([128, H], F32, tag="work")
                    pa = paw[:, :128]
                    for hc in range(HC):
                        nc.tensor.matmul(
                            pa[:],
                            lhsT=wa_sb[:, hc, ac * 128 : (ac + 1) * 128],
                            rhs=cwrT[:, hc, 1:J],
                            start=(hc == 0),
                            stop=(hc == HC - 1),
                        )
                    nc.vector.tensor_scalar(
                        out=haT[:, ac, :],
                        in0=pa[:],
                        scalar1=pkf_sb[:, 2 + ac : 3 + ac],
                        scalar2=None,
                        op0=ALU.add,
                    )
                # ---- cbb_T [128, 2, 129] f32 ----
                cbbT = bpool.tile([128, HC, J], F32, tag="cbbT")
                for bc in range(HC):
                    pc = ps_big.tile([128, J], F32, tag="pcb")
                    for hc in range(HC):
                        nc.tensor.matmul(
                            pc[:],
                            lhsT=wb_sb[:, hc, bc * 128 : (bc + 1) * 128],
                            rhs=cwrT[:, hc, :],
                            start=(hc == 0),
                            stop=(hc == HC - 1),
                        )
                    nc.scalar.copy(cbbT[:, bc, :], pc[:])
                # ---- cbb in [j, h] layout (bf16, for the gold-select matmul) ----
                cj = bpool.tile([128, H], BF16, tag="cj")
                pj = ps_work.tile([128, H], F32, tag="work")
                for hc in range(HC):
                    nc.tensor.matmul(
                        pj[:],
                        lhsT=cwrT[:, hc, 0:128],
                        rhs=wb_sb[:, hc, :],
                        start=(hc == 0),
                        stop=(hc == HC - 1),
                    )
                nc.scalar.copy(cj[:], pj[:])
                cjl = bpool.tile([1, H], BF16, tag="cjl")
                pjlw = ps_work.tile([128, H], F32, tag="work")
                pjl = pjlw[0:1, :]
                for hc in range(HC):
                    nc.tensor.matmul(
                        pjl[:],
                        lhsT=cwrT[:, hc, 128:129],
                        rhs=wb_sb[:, hc, :],
                        start=(hc == 0),
                        stop=(hc == HC - 1),
                    )
                nc.scalar.copy(cjl[:], pjl[:])

                # ---- label path: sel_pre = ha + cbb[gold] via PE (all-SBUF) ----
                ohb = oh_sb[:, (2 * b) * 128 : (2 * b + 1) * 128]
                ohr = oh_sb[0:1, (2 * b + 1) * 128 : (2 * b + 2) * 128]
                selT = bpool.tile([128, HC, 128], BF16, tag="selT")
                for hc in range(HC):
                    pselw = ps_work.tile([128, H], F32, tag="work")
                    psel = pselw[:, :128]
                    nc.tensor.matmul(
                        psel[:], lhsT=ident_sb[:], rhs=haT[:, hc, :],
                        start=True, stop=False,
                    )
                    nc.tensor.matmul(
                        psel[:], lhsT=cj[:, hc * 128 : (hc + 1) * 128], rhs=ohb,
                        start=False, stop=False,
                    )
                    nc.tensor.matmul(
                        psel[:], lhsT=cjl[0:1, hc * 128 : (hc + 1) * 128], rhs=ohr,
                        start=False, stop=True,
                    )
                    nc.vector.tensor_scalar(
                        out=selT[:, hc, :], in0=psel[:], scalar1=0.0, op0=ALU.max,
                        scalar2=None,
                    )
                lab_ps = ps_lab.tile([128, TAGS], F32, tag="lab")
                for hc in range(HC):
                    nc.tensor.matmul(
                        lab_ps[:],
                        lhsT=selT[:, hc, :],
                        rhs=pkb_sb[:, 4 + TAGS * hc : 4 + TAGS * (hc + 1)],
                        start=(hc == 0),
                        stop=False,
                    )
                nc.tensor.matmul(
                    lab_ps[:], lhsT=ones1[:], rhs=pkf_sb[0:1, 8 + J : 8 + J + TAGS],
                    start=False, stop=True,
                )
                # ---- the quadratic j-loop ----
                arc_ps = ps_big.tile([128, J], F32, tag="arc")
                for j in range(J):
                    for hc in range(HC):
                        pt = pairs_pool.tile([128, 128], BF16, tag="pairs")
                        eng = SPLIT40[(j * HC + hc) % 40]
                        if eng == "a":
                            nc.scalar.activation(
                                pt[:],
                                haT[:, hc, :],
                                AF.Relu,
                                bias=cbbT[:, hc, j : j + 1],
                            )
                        else:
                            veng = nc.vector if eng == "d" else nc.gpsimd
                            veng.tensor_scalar(
                                out=pt[:],
                                in0=haT[:, hc, :],
                                scalar1=cbbT[:, hc, j : j + 1],
                                scalar2=0.0,
                                op0=ALU.add,
                                op1=ALU.max,
                            )
                        nc.tensor.matmul(
                            arc_ps[:, j : j + 1],
                            lhsT=pt[:],
                            rhs=pkb_sb[:, 2 + hc : 3 + hc],
                            start=(hc == 0),
                            stop=(hc == HC - 1),
                        )

                # ---- per-example CE pieces (overlap the next example) ----
                eta = bpool.tile([128, J], F32, tag="eta")
                nc.scalar.activation(
                    eta[:], arc_ps[:], AF.Exp,
                    accum_out=es4[:, 2 * b : 2 * b + 1],
                )
                etl = bpool.tile([128, TAGS], F32, tag="etl")
                nc.scalar.activation(
                    etl[:], lab_ps[:], AF.Exp,
                    accum_out=es4[:, 2 * b + 1 : 2 * b + 2],
                )
                sc2 = bpool.tile([128, J], F32, tag="sc2")
                nc.vector.scalar_tensor_tensor(
                    out=sc2[:],
                    in0=pkf_sb[:, 8 : 8 + J],
                    scalar=pkf_sb[:, 4 + b : 5 + b],
                    op0=ALU.is_equal,
                    in1=arc_ps[:],
                    op1=ALU.mult,
                    accum_out=golds4[:, 2 * b : 2 * b + 1],
                )
                sc2l = bpool.tile([128, TAGS], F32, tag="sc2l")
                nc.vector.scalar_tensor_tensor(
                    out=sc2l[:],
                    in0=pkf_sb[:, 8 : 8 + TAGS],
                    scalar=pkf_sb[:, 6 + b : 7 + b],
                    op0=ALU.is_equal,
                    in1=lab_ps[:],
                    op1=ALU.mult,
                    accum_out=golds4[:, 2 * b + 1 : 2 * b + 2],
                )

            nc.sync.dma_start(out=esg_d.ap(), in_=esg[:])

    nc.compile()
    return nc


def _prep_in_maps(inputs):
    ctx = np.asarray(inputs["contextualized"], np.float32)
    arcs = np.asarray(inputs["desired_arcs"], np.int32)
    labs = np.asarray(inputs["desired_labels"], np.int32)
    W1 = np.asarray(inputs["W1"], np.float32)
    b1 = np.asarray(inputs["b1"], np.float32)
    root = np.asarray(inputs["root"], np.float32)
    Wp = np.asarray(inputs["Wp"], np.float32)
    bp = np.asarray(inputs["bp"], np.float32)
    W_arc = np.asarray(inputs["W_arc"], np.float32)
    W_lab = np.asarray(inputs["W_lab"], np.float32)
    b_lab = np.asarray(inputs["b_lab"], np.float32)

    def chunked(w, nch):  # [nch*128, X] -> [128, nch, X]
        return np.ascontiguousarray(
            w.reshape(nch, 128, -1).transpose(1, 0, 2)
        )

    w1_bf = chunked(W1, DC).astype(_nb)
    wab_bf = np.concatenate(
        [chunked(Wp[:H], HC), chunked(Wp[H:], HC)], axis=1
    ).astype(_nb)  # [128, 4, 256]: wa chunks then wb chunks

    pkb = np.zeros((128, 4 + HC * TAGS), np.float32)
    pkb[:, 0:2] = root.reshape(HC, 128).T
    pkb[:, 2:4] = W_arc[:, 0].reshape(HC, 128).T
    for hc in range(HC):
        pkb[:, 4 + TAGS * hc : 4 + TAGS * (hc + 1)] = W_lab[hc * 128 : (hc + 1) * 128]
    pkb = pkb.astype(_nb)

    pkf_base = np.zeros((128, PKF_W), np.float32)
    pkf_base[:, 0:2] = b1.reshape(HC, 128).T
    pkf_base[:, 2:4] = bp.reshape(HC, 128).T
    pkf_base[:, 8 : 8 + J] = np.arange(J, dtype=np.float32)[None, :]
    pkf_base[:, 8 + J : 8 + J + TAGS] = b_lab[None, :]

    jrange = np.arange(128, dtype=np.int32)
    in_maps = []
    for c in range(NC_CORES):
        bs = slice(c * NB, (c + 1) * NB)
        arcs_c = arcs[bs]  # [NB, 128]
        pkf = pkf_base.copy()
        pkf[:, 4:6] = arcs_c.T.astype(np.float32)
        pkf[:, 6:8] = labs[bs].T.astype(np.float32)
        oh = np.zeros((128, NB * 2 * 128), np.float32)
        for b in range(NB):
            oh[:, (2 * b) * 128 : (2 * b + 1) * 128] = (
                jrange[:, None] == arcs_c[b][None, :]
            )
            oh[0, (2 * b + 1) * 128 : (2 * b + 2) * 128] = arcs_c[b] == 128
        in_maps.append(
            {
                "ctx_bf": np.ascontiguousarray(
                    ctx[bs].reshape(NB, L, DC, 128).transpose(0, 3, 2, 1)
                    .reshape(NB, 128, DC * 128)
                ).astype(_nb),
                "w1_bf": w1_bf,
                "wab_bf": wab_bf,
                "pkf_f32": pkf,
                "pack_bf": pkb,
                "oh_bf": oh.astype(_nb),
            }
        )
    return in_maps


def kernel(**inputs) -> np.ndarray:
    if "nc" not in _cached:
        _cached["nc"] = _build_program()
    nc = _cached["nc"]
    in_maps = _prep_in_maps(inputs)
    res = run_bass_kernel_spmd(nc, in_maps, list(range(NC_CORES)))
    # esg: [128, 4*NB] per core = es (sumexp) cols then gold-logit cols
    ce_parts = []
    for r in res.results:
        esg = r["esg_out"].astype(np.float64)  # [128, 4*NB]
        es = esg[:, 0 : 2 * NB]
        golds = esg[:, 2 * NB : 4 * NB]
        ce = np.log(es) - golds  # [128, 2*NB]: arc/lab interleaved per example
        for b in range(NB):
            ce_parts.append(ce[:, 2 * b] + ce[:, 2 * b + 1])  # [128]
    ce_all = np.stack(ce_parts, axis=1)  # [128, B]
    lens = np.asarray(inputs["sentence_lengths"], np.int32)  # [B]
    mask = (np.arange(L)[None, :] < lens[:, None]).astype(np.float64)  # [B, L]
    total = float(np.sum(ce_all.T * mask))
    denom = max(float(mask.sum()), 1.0)
    return np.array(0.5 * total / denom, dtype=np.float32)


# revision 20
# speedup vs baseline: 1.0054x; 1.0054x over previous
"""Trainium2 Bass kernel for nn_ModelDEP (biaffine-ish dependency parser loss).

Contract: kernel(**inputs) takes FULL unsharded numpy inputs (as produced by
reference.setup_inputs()) and returns the FULL output (scalar f32 loss).

Strategy (hardcoded, self-contained):
  - Data parallel over batch: B=16 examples -> 8 cores x 2 examples.
  - Per example, on device:
      hidden_T = relu(W1.T @ ctx_T + b1)            [256h x 128i] (h on partitions)
      cwr_T    = [root | hidden_T]                  [256h x 129j]
      ha_T     = Wa.T @ hidden_T + bp               [256 x 128]   (bp folded here)
      cbb_T    = Wb.T @ cwr_T                       [256 x 129]
      arc[i,j] = W_arc . relu(ha_T[:,i] + cbb_T[:,j])
        - per (j, h-chunk): one fused (add bias, max 0) op -> bf16 [128,128] tile
          split between DVE tensor_scalar (~180ns/tile effective) and ACT
          activation-relu (~341ns/tile effective), 26:14 greedy interleave.
          GPSIMD measured ~2µs/tile and regresses the kernel - do not use it.
        - TensorE: lhsT = pairs tile (stationary), rhs = W_arc chunk
          -> PSUM column [128i, 1], accumulated over the 2 h-chunks.
      arc CE: logits are tiny (|x| < 0.15) so no max-subtraction: one Exp with
        accum (sum over j); gold logit via (iota == gold) * logits.  The final
        ln(sum) - gold combine happens on host (tiny [128,8] tensor), so the
        kernel needs only Relu+Exp -> a single act-table load at start.
      label path (no DRAM round-trip): sel_pre[h,i] = ha[h,i] + cbb[gold_i,h]
        computed purely on PE: identity @ ha + cbb_jh(=cj) @ onehot(gold) +
        cbb_root @ onehot(gold==root), accumulated in PSUM; relu on DVE;
        label logits = sel_T.T @ W_lab + b_lab.  All-SBUF, so nothing ever
        blocks an in-order engine queue.
  - Few, large input DMAs (ctx as one [128, 512] transfer per example; weights
    as two [128, 4, 256] transfers on the gpsimd SWDGE queue).
  - Host: ln(es)-gold combine, mask by sentence length, global sum, /denom,
    *0.5.
"""

import sys
import numpy as np

for _p in ("/opt/trn_rl_repo", "/root/.axon_site/_ro/trn_rl_repo"):
    if _p not in sys.path:
        sys.path.append(_p)

import ml_dtypes

import concourse.bass as bass
from concourse import bacc
import concourse.mybir as mybir
import concourse.tile as tile
from concourse.bass_utils import run_bass_kernel_spmd
from concourse.masks import make_identity

BF16 = mybir.dt.bfloat16
F32 = mybir.dt.float32
AF = mybir.ActivationFunctionType
ALU = mybir.AluOpType

B, L, D, H, TAGS = 16, 128, 512, 256, 45
NC_CORES = 8
NB = B // NC_CORES  # examples per core
J = L + 1  # head candidates (root + tokens)
HC = H // 128  # h chunks
DC = D // 128  # d chunks

_nb = ml_dtypes.bfloat16

_cached = {}

# j-loop relu engine split by (j*HC+hc) % 40, assigned greedily by measured
# effective sustained rates (engine time + sem/queue overhead inflation).
def _make_split():
    # effective measured rates incl. per-engine overhead inflation
    rate = {"d": 180.0, "a": 341.0}
    t = {"d": 0.0, "a": 0.0}
    slots = []
    for _ in range(40):
        e = min(rate, key=lambda x: t[x] + rate[x])
        t[e] += rate[e]
        slots.append(e)
    return slots


SPLIT40 = _make_split()

PKF_W = 8 + J + TAGS  # pke (8) | iota row (J) | b_lab rows (TAGS)


def _build_program():
    nc = bacc.Bacc("TRN2", target_bir_lowering=False, debug=False, num_devices=NC_CORES)

    # ---- I/O ----
    ctx_d = nc.dram_tensor("ctx_bf", [NB, 128, DC * 128], BF16, kind="ExternalInput")
    w1_d = nc.dram_tensor("w1_bf", [128, DC, H], BF16, kind="ExternalInput")
    wab_d = nc.dram_tensor("wab_bf", [128, 2 * HC, H], BF16, kind="ExternalInput")
    pkf_d = nc.dram_tensor("pkf_f32", [128, PKF_W], F32, kind="ExternalInput")
    pkb_d = nc.dram_tensor("pack_bf", [128, 4 + HC * TAGS], BF16, kind="ExternalInput")
    oh_d = nc.dram_tensor("oh_bf", [128, NB * 2 * 128], BF16, kind="ExternalInput")
    esg_d = nc.dram_tensor("esg_out", [128, 4 * NB], F32, kind="ExternalOutput")

    with tile.TileContext(nc) as tc:
        with (
            tc.tile_pool(name="consts", bufs=1) as consts,
            tc.tile_pool(name="bpool", bufs=2) as bpool,
            tc.tile_pool(name="pairs", bufs=96) as pairs_pool,
            tc.tile_pool(name="ps_big", bufs=2, space="PSUM") as ps_big,
            tc.tile_pool(name="ps_work", bufs=2, space="PSUM") as ps_work,
            tc.tile_pool(name="ps_lab", bufs=2, space="PSUM") as ps_lab,
        ):
            # ---- critical-path DMAs: ctx ex0 (sync), weights (gpsimd SWDGE),
            #      packs (scalar) ----
            ctxTs = []
            ctxT0 = bpool.tile([128, DC, 128], BF16, tag="ctxT")
            nc.sync.dma_start(out=ctxT0[:], in_=ctx_d.ap()[0])
            w1_sb = consts.tile([128, DC, H], BF16)
            nc.gpsimd.dma_start(out=w1_sb[:], in_=w1_d.ap())
            wab_sb = consts.tile([128, 2 * HC, H], BF16)
            nc.gpsimd.dma_start(out=wab_sb[:], in_=wab_d.ap())
            pkf_sb = consts.tile([128, PKF_W], F32)
            nc.scalar.dma_start(out=pkf_sb[:], in_=pkf_d.ap())
            pkb_sb = consts.tile([128, 4 + HC * TAGS], BF16)
            nc.scalar.dma_start(out=pkb_sb[:], in_=pkb_d.ap())
            # non-critical loads
            ctxT1 = bpool.tile([128, DC, 128], BF16, tag="ctxT")
            nc.sync.dma_start(out=ctxT1[:], in_=ctx_d.ap()[1])
            oh_sb = consts.tile([128, NB * 2 * 128], BF16)
            nc.sync.dma_start(out=oh_sb[:], in_=oh_d.ap())
            ctxTs.append(ctxT0)
            ctxTs.append(ctxT1)

            # bf16 identity for the ha-into-PSUM matmul; ones row for b_lab
            ident_sb = consts.tile([128, 128], BF16)
            make_identity(nc, ident_sb[:])
            ones1 = consts.tile([1, 128], F32)
            nc.gpsimd.memset(ones1[:], 1.0)

            # pin the act table (exp_and_others: has Exp + Relu) before any Relu
            dummy = consts.tile([1, 1], F32)
            nc.scalar.activation(dummy[:], ones1[0:1, 0:1], AF.Exp)

            # output: es (sumexp) and gold logits, combined on host
            es4 = consts.tile([128, 2 * NB], F32)
            golds4 = consts.tile([128, 2 * NB], F32)

            wa_sb = wab_sb[:, 0:HC]
            wb_sb = wab_sb[:, HC : 2 * HC]

            for b in range(NB):
                ctxT = ctxTs[b]
                # ---- hidden (into cwr cols 1..128) ----
                cwrT = bpool.tile([128, HC, J], BF16, tag="cwrT")
                for hc in range(HC):
                    nc.vector.tensor_copy(cwrT[:, hc, 0:1], pkb_sb[:, hc : hc + 1])
                for hc in range(HC):
                    phw = ps_work.tile([128, H], F32, tag="work")
                    ph = phw[:, :128]
                    for dc in range(DC):
                        nc.tensor.matmul(
                            ph[:],
                            lhsT=w1_sb[:, dc, hc * 128 : (hc + 1) * 128],
                            rhs=ctxT[:, dc, :],
                            start=(dc == 0),
                            stop=(dc == DC - 1),
                        )
                    nc.vector.tensor_scalar(
                        out=cwrT[:, hc, 1:J],
                        in0=ph[:],
                        scalar1=pkf_sb[:, hc : hc + 1],
                        scalar2=0.0,
                        op0=ALU.add,
                        op1=ALU.max,
                    )
                # ---- ha_T (+bp folded) ----
                haT = bpool.tile([128, HC, 128], BF16, tag="haT")
                for ac in range(HC):
                    paw = ps_work.tile([128, H], F32, tag="work")
                    pa = paw[:, :128]
                    for hc in range(HC):
                        nc.tensor.matmul(
                            pa[:],
                            lhsT=wa_sb[:, hc, ac * 128 : (ac + 1) * 128],
                            rhs=cwrT[:, hc, 1:J],
                            start=(hc == 0),
                            stop=(hc == HC - 1),
                        )
                    nc.vector.tensor_scalar(
                        out=haT[:, ac, :],
                        in0=pa[:],
                        scalar1=pkf_sb[:, 2 + ac : 3 + ac],
                        scalar2=None,
                        op0=ALU.add,
                    )
                # ---- cbb_T [128, 2, 129] f32 ----
                cbbT = bpool.tile([128, HC, J], F32, tag="cbbT")
                for bc in range(HC):
                    pc = ps_big.tile([128, J], F32, tag="pcb")
                    for hc in range(HC):
                        nc.tensor.matmul(
                            pc[:],
                            lhsT=wb_sb[:, hc, bc * 128 : (bc + 1) * 128],
                            rhs=cwrT[:, hc, :],
                            start=(hc == 0),
                            stop=(hc == HC - 1),
                        )
                    nc.scalar.copy(cbbT[:, bc, :], pc[:])
                # ---- cbb in [j, h] layout (bf16, for the gold-select matmul) ----
                cj = bpool.tile([128, H], BF16, tag="cj")
                pj = ps_work.tile([128, H], F32, tag="work")
                for hc in range(HC):
                    nc.tensor.matmul(
                        pj[:],
                        lhsT=cwrT[:, hc, 0:128],
                        rhs=wb_sb[:, hc, :],
                        start=(hc == 0),
                        stop=(hc == HC - 1),
                    )
                nc.scalar.copy(cj[:], pj[:])
                cjl = bpool.tile([1, H], BF16, tag="cjl")
                pjlw = ps_work.tile([128, H], F32, tag="work")
                pjl = pjlw[0:1, :]
                for hc in range(HC):
                    nc.tensor.matmul(
                        pjl[:],
                        lhsT=cwrT[:, hc, 128:129],
                        rhs=wb_sb[:, hc, :],
                        start=(hc == 0),
                        stop=(hc == HC - 1),
                    )
                nc.scalar.copy(cjl[:], pjl[:])

                # ---- label path: sel_pre = ha + cbb[gold] via PE (all-SBUF) ----
                ohb = oh_sb[:, (2 * b) * 128 : (2 * b + 1) * 128]
                ohr = oh_sb[0:1, (2 * b + 1) * 128 : (2 * b + 2) * 128]
                selT = bpool.tile([128, HC, 128], BF16, tag="selT")
                for hc in range(HC):
                    pselw = ps_work.tile([128, H], F32, tag="work")
                    psel = pselw[:, :128]
                    nc.tensor.matmul(
                        psel[:], lhsT=ident_sb[:], rhs=haT[:, hc, :],
                        start=True, stop=False,
                    )
                    nc.tensor.matmul(
                        psel[:], lhsT=cj[:, hc * 128 : (hc + 1) * 128], rhs=ohb,
                        start=False, stop=False,
                    )
                    nc.tensor.matmul(
                        psel[:], lhsT=cjl[0:1, hc * 128 : (hc + 1) * 128], rhs=ohr,
                        start=False, stop=True,
                    )
                    nc.vector.tensor_scalar(
                        out=selT[:, hc, :], in0=psel[:], scalar1=0.0, op0=ALU.max,
                        scalar2=None,
                    )
                lab_ps = ps_lab.tile([128, TAGS], F32, tag="lab")
                for hc in range(HC):
                    nc.tensor.matmul(
                        lab_ps[:],
                        lhsT=selT[:, hc, :],
                        rhs=pkb_sb[:, 4 + TAGS * hc : 4 + TAGS * (hc + 1)],
                        start=(hc == 0),
                        stop=False,
                    )
                nc.tensor.matmul(
                    lab_ps[:], lhsT=ones1[:], rhs=pkf_sb[0:1, 8 + J : 8 + J + TAGS],
                    start=False, stop=True,
                )
                # ---- the quadratic j-loop ----
                arc_ps = ps_big.tile([128, J], F32, tag="arc")
                for j in range(J):
                    for hc in range(HC):
                        pt = pairs_pool.tile([128, 128], BF16, tag="pairs")
                        eng = SPLIT40[(j * HC + hc) % 40]
                        if eng == "a":
                            nc.scalar.activation(
                                pt[:],
                                haT[:, hc, :],
                                AF.Relu,
                                bias=cbbT[:, hc, j : j + 1],
                            )
                        else:
                            veng = nc.vector if eng == "d" else nc.gpsimd
                            veng.tensor_scalar(
                                out=pt[:],
                                in0=haT[:, hc, :],
                                scalar1=cbbT[:, hc, j : j + 1],
                                scalar2=0.0,
                                op0=ALU.add,
                                op1=ALU.max,
                            )
                        nc.tensor.matmul(
                            arc_ps[:, j : j + 1],
                            lhsT=pt[:],
                            rhs=pkb_sb[:, 2 + hc : 3 + hc],
                            start=(hc == 0),
                            stop=(hc == HC - 1),
                        )

                # ---- per-example CE pieces (overlap the next example).
                #      label side first: lab_ps is ready mid-loop, so only the
                #      arc side gates the tail. ----
                etl = bpool.tile([128, TAGS], F32, tag="etl")
                nc.scalar.activation(
                    etl[:], lab_ps[:], AF.Exp,
                    accum_out=es4[:, 2 * b + 1 : 2 * b + 2],
                )
                sc2l = bpool.tile([128, TAGS], F32, tag="sc2l")
                nc.vector.scalar_tensor_tensor(
                    out=sc2l[:],
                    in0=pkf_sb[:, 8 : 8 + TAGS],
                    scalar=pkf_sb[:, 6 + b : 7 + b],
                    op0=ALU.is_equal,
                    in1=lab_ps[:],
                    op1=ALU.mult,
                    accum_out=golds4[:, 2 * b + 1 : 2 * b + 2],
                )
                eta = bpool.tile([128, J], F32, tag="eta")
                nc.scalar.activation(
                    eta[:], arc_ps[:], AF.Exp,
                    accum_out=es4[:, 2 * b : 2 * b + 1],
                )
                sc2 = bpool.tile([128, J], F32, tag="sc2")
                nc.vector.scalar_tensor_tensor(
                    out=sc2[:],
                    in0=pkf_sb[:, 8 : 8 + J],
                    scalar=pkf_sb[:, 4 + b : 5 + b],
                    op0=ALU.is_equal,
                    in1=arc_ps[:],
                    op1=ALU.mult,
                    accum_out=golds4[:, 2 * b : 2 * b + 1],
                )

            nc.sync.dma_start(out=esg_d.ap()[:, 0 : 2 * NB], in_=es4[:])
            nc.sync.dma_start(out=esg_d.ap()[:, 2 * NB : 4 * NB], in_=golds4[:])

    nc.compile()
    return nc


def _prep_in_maps(inputs):
    ctx = np.asarray(inputs["contextualized"], np.float32)
    arcs = np.asarray(inputs["desired_arcs"], np.int32)
    labs = np.asarray(inputs["desired_labels"], np.int32)
    W1 = np.asarray(inputs["W1"], np.float32)
    b1 = np.asarray(inputs["b1"], np.float32)
    root = np.asarray(inputs["root"], np.float32)
    Wp = np.asarray(inputs["Wp"], np.float32)
    bp = np.asarray(inputs["bp"], np.float32)
    W_arc = np.asarray(inputs["W_arc"], np.float32)
    W_lab = np.asarray(inputs["W_lab"], np.float32)
    b_lab = np.asarray(inputs["b_lab"], np.float32)

    def chunked(w, nch):  # [nch*128, X] -> [128, nch, X]
        return np.ascontiguousarray(
            w.reshape(nch, 128, -1).transpose(1, 0, 2)
        )

    w1_bf = chunked(W1, DC).astype(_nb)
    wab_bf = np.concatenate(
        [chunked(Wp[:H], HC), chunked(Wp[H:], HC)], axis=1
    ).astype(_nb)  # [128, 4, 256]: wa chunks then wb chunks

    pkb = np.zeros((128, 4 + HC * TAGS), np.float32)
    pkb[:, 0:2] = root.reshape(HC, 128).T
    pkb[:, 2:4] = W_arc[:, 0].reshape(HC, 128).T
    for hc in range(HC):
        pkb[:, 4 + TAGS * hc : 4 + TAGS * (hc + 1)] = W_lab[hc * 128 : (hc + 1) * 128]
    pkb = pkb.astype(_nb)

    pkf_base = np.zeros((128, PKF_W), np.float32)
    pkf_base[:, 0:2] = b1.reshape(HC, 128).T
    pkf_base[:, 2:4] = bp.reshape(HC, 128).T
    pkf_base[:, 8 : 8 + J] = np.arange(J, dtype=np.float32)[None, :]
    pkf_base[:, 8 + J : 8 + J + TAGS] = b_lab[None, :]

    jrange = np.arange(128, dtype=np.int32)
    in_maps = []
    for c in range(NC_CORES):
        bs = slice(c * NB, (c + 1) * NB)
        arcs_c = arcs[bs]  # [NB, 128]
        pkf = pkf_base.copy()
        pkf[:, 4:6] = arcs_c.T.astype(np.float32)
        pkf[:, 6:8] = labs[bs].T.astype(np.float32)
        oh = np.zeros((128, NB * 2 * 128), np.float32)
        for b in range(NB):
            oh[:, (2 * b) * 128 : (2 * b + 1) * 128] = (
                jrange[:, None] == arcs_c[b][None, :]
            )
            oh[0, (2 * b + 1) * 128 : (2 * b + 2) * 128] = arcs_c[b] == 128
        in_maps.append(
            {
                "ctx_bf": np.ascontiguousarray(
                    ctx[bs].reshape(NB, L, DC, 128).transpose(0, 3, 2, 1)
                    .reshape(NB, 128, DC * 128)
                ).astype(_nb),
                "w1_bf": w1_bf,
                "wab_bf": wab_bf,
                "pkf_f32": pkf,
                "pack_bf": pkb,
                "oh_bf": oh.astype(_nb),
            }
        )
    return in_maps


def kernel(**inputs) -> np.ndarray:
    if "nc" not in _cached:
        _cached["nc"] = _build_program()
    nc = _cached["nc"]
    in_maps = _prep_in_maps(inputs)
    res = run_bass_kernel_spmd(nc, in_maps, list(range(NC_CORES)))
    # esg: [128, 4*NB] per core = es (sumexp) cols then gold-logit cols
    ce_parts = []
    for r in res.results:
        esg = r["esg_out"].astype(np.float64)  # [128, 4*NB]
        es = esg[:, 0 : 2 * NB]
        golds = esg[:, 2 * NB : 4 * NB]
        ce = np.log(es) - golds  # [128, 2*NB]: arc/lab interleaved per example
        for b in range(NB):
            ce_parts.append(ce[:, 2 * b] + ce[:, 2 * b + 1])  # [128]
    ce_all = np.stack(ce_parts, axis=1)  # [128, B]
    lens = np.asarray(inputs["sentence_lengths"], np.int32)  # [B]
    mask = (np.arange(L)[None, :] < lens[:, None]).astype(np.float64)  # [B, L]
    total = float(np.sum(ce_all.T * mask))
    denom = max(float(mask.sum()), 1.0)
    return np.array(0.5 * total / denom, dtype=np.float32)


# revision 21
# speedup vs baseline: 1.0114x; 1.0060x over previous
"""Trainium2 Bass kernel for nn_ModelDEP (biaffine-ish dependency parser loss).

Contract: kernel(**inputs) takes FULL unsharded numpy inputs (as produced by
reference.setup_inputs()) and returns the FULL output (scalar f32 loss).

Strategy (hardcoded, self-contained):
  - Data parallel over batch: B=16 examples -> 8 cores x 2 examples.
  - Per example, on device:
      hidden_T = relu(W1.T @ ctx_T + b1)            [256h x 128i] (h on partitions)
      cwr_T    = [root | hidden_T]                  [256h x 129j]
      ha_T     = Wa.T @ hidden_T + bp               [256 x 128]   (bp folded here)
      cbb_T    = Wb.T @ cwr_T                       [256 x 129]
      arc[i,j] = W_arc . relu(ha_T[:,i] + cbb_T[:,j])
        - per (j, h-chunk): one fused (add bias, max 0) op -> bf16 [128,128] tile
          split between DVE tensor_scalar (~180ns/tile effective) and ACT
          activation-relu (~341ns/tile effective), 26:14 greedy interleave.
          GPSIMD measured ~2µs/tile and regresses the kernel - do not use it.
        - TensorE: lhsT = pairs tile (stationary), rhs = W_arc chunk
          -> PSUM column [128i, 1], accumulated over the 2 h-chunks.
      arc CE: logits are tiny (|x| < 0.15) so no max-subtraction: one Exp with
        accum (sum over j); gold logit via (iota == gold) * logits.  The final
        ln(sum) - gold combine happens on host (tiny [128,8] tensor), so the
        kernel needs only Relu+Exp -> a single act-table load at start.
      label path (no DRAM round-trip): sel_pre[h,i] = ha[h,i] + cbb[gold_i,h]
        computed purely on PE: identity @ ha + cbb_jh(=cj) @ onehot(gold) +
        cbb_root @ onehot(gold==root), accumulated in PSUM; relu on DVE;
        label logits = sel_T.T @ W_lab + b_lab.  All-SBUF, so nothing ever
        blocks an in-order engine queue.
  - Few, large input DMAs (ctx as one [128, 512] transfer per example; weights
    as two [128, 4, 256] transfers on the gpsimd SWDGE queue).
  - Host: ln(es)-gold combine, mask by sentence length, global sum, /denom,
    *0.5.
"""

import sys
import numpy as np

for _p in ("/opt/trn_rl_repo", "/root/.axon_site/_ro/trn_rl_repo"):
    if _p not in sys.path:
        sys.path.append(_p)

import ml_dtypes

import concourse.bass as bass
from concourse import bacc
import concourse.mybir as mybir
import concourse.tile as tile
from concourse.bass_utils import run_bass_kernel_spmd
from concourse.masks import make_identity

BF16 = mybir.dt.bfloat16
F32 = mybir.dt.float32
AF = mybir.ActivationFunctionType
ALU = mybir.AluOpType

B, L, D, H, TAGS = 16, 128, 512, 256, 45
NC_CORES = 8
NB = B // NC_CORES  # examples per core
J = L + 1  # head candidates (root + tokens)
HC = H // 128  # h chunks
DC = D // 128  # d chunks

_nb = ml_dtypes.bfloat16

_cached = {}

# j-loop relu engine split by (j*HC+hc) % 40, assigned greedily by measured
# effective sustained rates (engine time + sem/queue overhead inflation).
def _make_split():
    # effective measured rates incl. per-engine overhead inflation
    rate = {"d": 180.0, "a": 341.0}
    t = {"d": 0.0, "a": 0.0}
    slots = []
    for _ in range(40):
        e = min(rate, key=lambda x: t[x] + rate[x])
        t[e] += rate[e]
        slots.append(e)
    return slots


SPLIT40 = _make_split()

PKF_W = 8 + J + TAGS  # pke (8) | iota row (J) | b_lab rows (TAGS)


def _build_program():
    nc = bacc.Bacc("TRN2", target_bir_lowering=False, debug=False, num_devices=NC_CORES)

    # ---- I/O ----
    ctx_d = nc.dram_tensor("ctx_bf", [NB, 128, DC * 128], BF16, kind="ExternalInput")
    w1_d = nc.dram_tensor("w1_bf", [128, DC, H], BF16, kind="ExternalInput")
    wab_d = nc.dram_tensor("wab_bf", [128, 2 * HC, H], BF16, kind="ExternalInput")
    pkf_d = nc.dram_tensor("pkf_f32", [128, PKF_W], F32, kind="ExternalInput")
    pkb_d = nc.dram_tensor("pack_bf", [128, 4 + HC * TAGS], BF16, kind="ExternalInput")
    oh_d = nc.dram_tensor("oh_bf", [128, NB * 2 * 128], BF16, kind="ExternalInput")
    esg_d = nc.dram_tensor("esg_out", [128, 4 * NB], F32, kind="ExternalOutput")

    with tile.TileContext(nc) as tc:
        with (
            tc.tile_pool(name="consts", bufs=1) as consts,
            tc.tile_pool(name="bpool", bufs=2) as bpool,
            tc.tile_pool(name="pairs", bufs=96) as pairs_pool,
            tc.tile_pool(name="ps_big", bufs=2, space="PSUM") as ps_big,
            tc.tile_pool(name="ps_work", bufs=2, space="PSUM") as ps_work,
            tc.tile_pool(name="ps_lab", bufs=2, space="PSUM") as ps_lab,
        ):
            # ---- critical-path DMAs: ctx ex0 (sync), weights (gpsimd SWDGE),
            #      packs (scalar) ----
            ctxTs = []
            ctxT0 = bpool.tile([128, DC, 128], BF16, tag="ctxT")
            nc.sync.dma_start(out=ctxT0[:], in_=ctx_d.ap()[0])
            w1_sb = consts.tile([128, DC, H], BF16)
            nc.gpsimd.dma_start(out=w1_sb[:], in_=w1_d.ap())
            wab_sb = consts.tile([128, 2 * HC, H], BF16)
            nc.gpsimd.dma_start(out=wab_sb[:], in_=wab_d.ap())
            pkf_sb = consts.tile([128, PKF_W], F32)
            nc.scalar.dma_start(out=pkf_sb[:], in_=pkf_d.ap())
            pkb_sb = consts.tile([128, 4 + HC * TAGS], BF16)
            nc.scalar.dma_start(out=pkb_sb[:], in_=pkb_d.ap())
            # non-critical loads
            ctxT1 = bpool.tile([128, DC, 128], BF16, tag="ctxT")
            nc.sync.dma_start(out=ctxT1[:], in_=ctx_d.ap()[1])
            oh_sb = consts.tile([128, NB * 2 * 128], BF16)
            nc.sync.dma_start(out=oh_sb[:], in_=oh_d.ap())
            ctxTs.append(ctxT0)
            ctxTs.append(ctxT1)

            # bf16 identity for the ha-into-PSUM matmul; ones row for b_lab
            ident_sb = consts.tile([128, 128], BF16)
            make_identity(nc, ident_sb[:])
            ones1 = consts.tile([1, 128], F32)
            nc.gpsimd.memset(ones1[:], 1.0)

            # pin the act table (exp_and_others: has Exp + Relu) before any Relu
            dummy = consts.tile([1, 1], F32)
            nc.scalar.activation(dummy[:], ones1[0:1, 0:1], AF.Exp)

            # output: es (sumexp) and gold logits, combined on host
            es4 = consts.tile([128, 2 * NB], F32)
            golds4 = consts.tile([128, 2 * NB], F32)

            wa_sb = wab_sb[:, 0:HC]
            wb_sb = wab_sb[:, HC : 2 * HC]

            for b in range(NB):
                ctxT = ctxTs[b]
                # ---- hidden (into cwr cols 1..128) ----
                cwrT = bpool.tile([128, HC, J], BF16, tag="cwrT")
                for hc in range(HC):
                    nc.vector.tensor_copy(cwrT[:, hc, 0:1], pkb_sb[:, hc : hc + 1])
                for hc in range(HC):
                    phw = ps_work.tile([128, H], F32, tag="work")
                    ph = phw[:, :128]
                    for dc in range(DC):
                        nc.tensor.matmul(
                            ph[:],
                            lhsT=w1_sb[:, dc, hc * 128 : (hc + 1) * 128],
                            rhs=ctxT[:, dc, :],
                            start=(dc == 0),
                            stop=(dc == DC - 1),
                        )
                    nc.vector.tensor_scalar(
                        out=cwrT[:, hc, 1:J],
                        in0=ph[:],
                        scalar1=pkf_sb[:, hc : hc + 1],
                        scalar2=0.0,
                        op0=ALU.add,
                        op1=ALU.max,
                    )
                # ---- ha_T (+bp folded) ----
                haT = bpool.tile([128, HC, 128], BF16, tag="haT")
                for ac in range(HC):
                    paw = ps_work.tile([128, H], F32, tag="work")
                    pa = paw[:, :128]
                    for hc in range(HC):
                        nc.tensor.matmul(
                            pa[:],
                            lhsT=wa_sb[:, hc, ac * 128 : (ac + 1) * 128],
                            rhs=cwrT[:, hc, 1:J],
                            start=(hc == 0),
                            stop=(hc == HC - 1),
                        )
                    nc.vector.tensor_scalar(
                        out=haT[:, ac, :],
                        in0=pa[:],
                        scalar1=pkf_sb[:, 2 + ac : 3 + ac],
                        scalar2=None,
                        op0=ALU.add,
                    )
                # ---- cbb_T [128, 2, 129] f32 ----
                cbbT = bpool.tile([128, HC, J], F32, tag="cbbT")
                for bc in range(HC):
                    pc = ps_big.tile([128, J], F32, tag="pcb")
                    for hc in range(HC):
                        nc.tensor.matmul(
                            pc[:],
                            lhsT=wb_sb[:, hc, bc * 128 : (bc + 1) * 128],
                            rhs=cwrT[:, hc, :],
                            start=(hc == 0),
                            stop=(hc == HC - 1),
                        )
                    nc.scalar.copy(cbbT[:, bc, :], pc[:])
                # ---- cbb in [j, h] layout (bf16, for the gold-select matmul) ----
                cj = bpool.tile([128, H], BF16, tag="cj")
                pj = ps_work.tile([128, H], F32, tag="work")
                for hc in range(HC):
                    nc.tensor.matmul(
                        pj[:],
                        lhsT=cwrT[:, hc, 0:128],
                        rhs=wb_sb[:, hc, :],
                        start=(hc == 0),
                        stop=(hc == HC - 1),
                    )
                nc.scalar.copy(cj[:], pj[:])
                cjl = bpool.tile([1, H], BF16, tag="cjl")
                pjlw = ps_work.tile([128, H], F32, tag="work")
                pjl = pjlw[0:1, :]
                for hc in range(HC):
                    nc.tensor.matmul(
                        pjl[:],
                        lhsT=cwrT[:, hc, 128:129],
                        rhs=wb_sb[:, hc, :],
                        start=(hc == 0),
                        stop=(hc == HC - 1),
                    )
                nc.scalar.copy(cjl[:], pjl[:])

                # ---- label path: sel_pre = ha + cbb[gold] via PE (all-SBUF) ----
                ohb = oh_sb[:, (2 * b) * 128 : (2 * b + 1) * 128]
                ohr = oh_sb[0:1, (2 * b + 1) * 128 : (2 * b + 2) * 128]
                selT = bpool.tile([128, HC, 128], BF16, tag="selT")
                for hc in range(HC):
                    pselw = ps_work.tile([128, H], F32, tag="work")
                    psel = pselw[:, :128]
                    nc.tensor.matmul(
                        psel[:], lhsT=ident_sb[:], rhs=haT[:, hc, :],
                        start=True, stop=False,
                    )
                    nc.tensor.matmul(
                        psel[:], lhsT=cj[:, hc * 128 : (hc + 1) * 128], rhs=ohb,
                        start=False, stop=False,
                    )
                    nc.tensor.matmul(
                        psel[:], lhsT=cjl[0:1, hc * 128 : (hc + 1) * 128], rhs=ohr,
                        start=False, stop=True,
                    )
                    nc.vector.tensor_scalar(
                        out=selT[:, hc, :], in0=psel[:], scalar1=0.0, op0=ALU.max,
                        scalar2=None,
                    )
                lab_ps = ps_lab.tile([128, TAGS], F32, tag="lab")
                for hc in range(HC):
                    nc.tensor.matmul(
                        lab_ps[:],
                        lhsT=selT[:, hc, :],
                        rhs=pkb_sb[:, 4 + TAGS * hc : 4 + TAGS * (hc + 1)],
                        start=(hc == 0),
                        stop=False,
                    )
                nc.tensor.matmul(
                    lab_ps[:], lhsT=ones1[:], rhs=pkf_sb[0:1, 8 + J : 8 + J + TAGS],
                    start=False, stop=True,
                )
                # ---- the quadratic j-loop ----
                arc_ps = ps_big.tile([128, J], F32, tag="arc")
                for j in range(J):
                    for hc in range(HC):
                        pt = pairs_pool.tile([128, 128], BF16, tag="pairs")
                        eng = SPLIT40[(j * HC + hc) % 40]
                        if eng == "a":
                            nc.scalar.activation(
                                pt[:],
                                haT[:, hc, :],
                                AF.Relu,
                                bias=cbbT[:, hc, j : j + 1],
                            )
                        else:
                            veng = nc.vector if eng == "d" else nc.gpsimd
                            veng.tensor_scalar(
                                out=pt[:],
                                in0=haT[:, hc, :],
                                scalar1=cbbT[:, hc, j : j + 1],
                                scalar2=0.0,
                                op0=ALU.add,
                                op1=ALU.max,
                            )
                        nc.tensor.matmul(
                            arc_ps[:, j : j + 1],
                            lhsT=pt[:],
                            rhs=pkb_sb[:, 2 + hc : 3 + hc],
                            start=(hc == 0),
                            stop=(hc == HC - 1),
                        )

                # ---- per-example CE pieces (overlap the next example).
                #      label side first: lab_ps is ready mid-loop, so only the
                #      arc side gates the tail. ----
                etl = bpool.tile([128, TAGS], F32, tag="etl")
                nc.scalar.activation(
                    etl[:], lab_ps[:], AF.Exp,
                    accum_out=es4[:, 2 * b + 1 : 2 * b + 2],
                )
                sc2l = bpool.tile([128, TAGS], F32, tag="sc2l")
                nc.vector.scalar_tensor_tensor(
                    out=sc2l[:],
                    in0=pkf_sb[:, 8 : 8 + TAGS],
                    scalar=pkf_sb[:, 6 + b : 7 + b],
                    op0=ALU.is_equal,
                    in1=lab_ps[:],
                    op1=ALU.mult,
                    accum_out=golds4[:, 2 * b + 1 : 2 * b + 2],
                )
                eta = bpool.tile([128, J], F32, tag="eta")
                nc.scalar.activation(
                    eta[:], arc_ps[:], AF.Exp,
                    accum_out=es4[:, 2 * b : 2 * b + 1],
                )
                sc2 = bpool.tile([128, J], F32, tag="sc2")
                nc.vector.scalar_tensor_tensor(
                    out=sc2[:],
                    in0=pkf_sb[:, 8 : 8 + J],
                    scalar=pkf_sb[:, 4 + b : 5 + b],
                    op0=ALU.is_equal,
                    in1=arc_ps[:],
                    op1=ALU.mult,
                    accum_out=golds4[:, 2 * b : 2 * b + 1],
                )

            nc.scalar.dma_start(out=esg_d.ap()[:, 0 : 2 * NB], in_=es4[:])
            nc.sync.dma_start(out=esg_d.ap()[:, 2 * NB : 4 * NB], in_=golds4[:])

    nc.compile()
    return nc


def _prep_in_maps(inputs):
    ctx = np.asarray(inputs["contextualized"], np.float32)
    arcs = np.asarray(inputs["desired_arcs"], np.int32)
    labs = np.asarray(inputs["desired_labels"], np.int32)
    W1 = np.asarray(inputs["W1"], np.float32)
    b1 = np.asarray(inputs["b1"], np.float32)
    root = np.asarray(inputs["root"], np.float32)
    Wp = np.asarray(inputs["Wp"], np.float32)
    bp = np.asarray(inputs["bp"], np.float32)
    W_arc = np.asarray(inputs["W_arc"], np.float32)
    W_lab = np.asarray(inputs["W_lab"], np.float32)
    b_lab = np.asarray(inputs["b_lab"], np.float32)

    def chunked(w, nch):  # [nch*128, X] -> [128, nch, X]
        return np.ascontiguousarray(
            w.reshape(nch, 128, -1).transpose(1, 0, 2)
        )

    w1_bf = chunked(W1, DC).astype(_nb)
    wab_bf = np.concatenate(
        [chunked(Wp[:H], HC), chunked(Wp[H:], HC)], axis=1
    ).astype(_nb)  # [128, 4, 256]: wa chunks then wb chunks

    pkb = np.zeros((128, 4 + HC * TAGS), np.float32)
    pkb[:, 0:2] = root.reshape(HC, 128).T
    pkb[:, 2:4] = W_arc[:, 0].reshape(HC, 128).T
    for hc in range(HC):
        pkb[:, 4 + TAGS * hc : 4 + TAGS * (hc + 1)] = W_lab[hc * 128 : (hc + 1) * 128]
    pkb = pkb.astype(_nb)

    pkf_base = np.zeros((128, PKF_W), np.float32)
    pkf_base[:, 0:2] = b1.reshape(HC, 128).T
    pkf_base[:, 2:4] = bp.reshape(HC, 128).T
    pkf_base[:, 8 : 8 + J] = np.arange(J, dtype=np.float32)[None, :]
    pkf_base[:, 8 + J : 8 + J + TAGS] = b_lab[None, :]

    jrange = np.arange(128, dtype=np.int32)
    in_maps = []
    for c in range(NC_CORES):
        bs = slice(c * NB, (c + 1) * NB)
        arcs_c = arcs[bs]  # [NB, 128]
        pkf = pkf_base.copy()
        pkf[:, 4:6] = arcs_c.T.astype(np.float32)
        pkf[:, 6:8] = labs[bs].T.astype(np.float32)
        oh = np.zeros((128, NB * 2 * 128), np.float32)
        for b in range(NB):
            oh[:, (2 * b) * 128 : (2 * b + 1) * 128] = (
                jrange[:, None] == arcs_c[b][None, :]
            )
            oh[0, (2 * b + 1) * 128 : (2 * b + 2) * 128] = arcs_c[b] == 128
        in_maps.append(
            {
                "ctx_bf": np.ascontiguousarray(
                    ctx[bs].reshape(NB, L, DC, 128).transpose(0, 3, 2, 1)
                    .reshape(NB, 128, DC * 128)
                ).astype(_nb),
                "w1_bf": w1_bf,
                "wab_bf": wab_bf,
                "pkf_f32": pkf,
                "pack_bf": pkb,
                "oh_bf": oh.astype(_nb),
            }
        )
    return in_maps


def kernel(**inputs) -> np.ndarray:
    if "nc" not in _cached:
        _cached["nc"] = _build_program()
    nc = _cached["nc"]
    in_maps = _prep_in_maps(inputs)
    res = run_bass_kernel_spmd(nc, in_maps, list(range(NC_CORES)))
    # esg: [128, 4*NB] per core = es (sumexp) cols then gold-logit cols
    ce_parts = []
    for r in res.results:
        esg = r["esg_out"].astype(np.float64)  # [128, 4*NB]
        es = esg[:, 0 : 2 * NB]
        golds = esg[:, 2 * NB : 4 * NB]
        ce = np.log(es) - golds  # [128, 2*NB]: arc/lab interleaved per example
        for b in range(NB):
            ce_parts.append(ce[:, 2 * b] + ce[:, 2 * b + 1])  # [128]
    ce_all = np.stack(ce_parts, axis=1)  # [128, B]
    lens = np.asarray(inputs["sentence_lengths"], np.int32)  # [B]
    mask = (np.arange(L)[None, :] < lens[:, None]).astype(np.float64)  # [B, L]
    total = float(np.sum(ce_all.T * mask))
    denom = max(float(mask.sum()), 1.0)
    return np.array(0.5 * total / denom, dtype=np.float32)
